# revision 1
# baseline (speedup 1.0000x reference)
"""Trainium2 Bass kernel for nn_CAM_62852551409742.

Math (reference):
  f = feats[:, :, 0, :]                               [R,B,T], R=4, B=512, T=150
  feat_n = feats.reshape(B, R*T)                      [B,K], K=600
  att[r,b,t,k] = tanh(a[r]*f[r,b,t] * feat_n[b,k])
  Hm = relu(att @ Wc[r].T + f*W[r])                   [R,B,T,32]
  attf = Hm @ Wh[r] + f                               [R,B,T]
  ff[b, r*T+t] = attf[r,b,t]
  out = (ff @ W1.T + b1) @ W2.T + b2                  [B,1,7]

Strategy: data-parallel over B across 8 cores (64 batches each). On device,
per 8-batch group: DVE builds z[k,(b,r,t)] = af broadcast * fn column
(tensor_scalar, 4x bf16), ACT applies tanh in place with huge free dims,
PE contracts k against Wc^T tiles into PSUM [(r,c) x (b,t)] chunks
(f*W folded in as an extra contraction row on the last k-tile), DVE relu ->
Hm_all bf16. Final: the linear tail is algebraically collapsed on the host
(Wx = W2@W1, U[(r,c),t,i] = Wh[r,c]*Wx[i,r*T+t]) so 150 small matmuls
(lhsT = Hm slice, rhs = U_t) plus 5 fp32 matmuls (f^T x Wx^T) accumulate the
final [64,7] directly in PSUM.
"""

from contextlib import ExitStack

import numpy as np
import ml_dtypes

import concourse.bacc as bacc
import concourse.bass as bass
import concourse.tile as tile
from concourse import mybir
from concourse import bass_utils

R, B, T, H = 4, 512, 150, 32
K = R * T                      # 600
NCORES = 8
BL = B // NCORES               # 64 batches per core
GB = 10                        # max batches per group (tile sizing)
KTS = [(0, 128), (128, 128), (256, 128), (384, 128), (512, 88)]
F32 = mybir.dt.float32
BF16 = mybir.dt.bfloat16
BF = ml_dtypes.bfloat16

_CACHE = {}


def build_nc():
    nc = bacc.Bacc("TRN2", target_bir_lowering=False)
    af_d = nc.dram_tensor("af", [BL, K], BF16, kind="ExternalInput")
    f_d = nc.dram_tensor("fr", [1, BL, K], BF16, kind="ExternalInput")
    fn_d = nc.dram_tensor("fn", [128, 5, BL], F32, kind="ExternalInput")
    wc_d = nc.dram_tensor("wc", [128, R, 5, H], BF16, kind="ExternalInput")
    u_d = nc.dram_tensor("u", [128, T, 7], BF16, kind="ExternalInput")
    ft_d = nc.dram_tensor("ft", [128, 5, BL], F32, kind="ExternalInput")
    wx_d = nc.dram_tensor("wx", [128, 5, 7], F32, kind="ExternalInput")
    bx_d = nc.dram_tensor("bx", [7, 1], F32, kind="ExternalInput")
    out_d = nc.dram_tensor("out", [7, BL], F32, kind="ExternalOutput")

    with tile.TileContext(nc) as tc, ExitStack() as ctx:
        consts = ctx.enter_context(tc.tile_pool(name="consts", bufs=1))
        attp = ctx.enter_context(tc.tile_pool(name="att", bufs=2))
        afp = ctx.enter_context(tc.tile_pool(name="afp", bufs=2))
        hmp = ctx.enter_context(tc.tile_pool(name="hm", bufs=1))
        outp = ctx.enter_context(tc.tile_pool(name="outp", bufs=1))
        psum = ctx.enter_context(tc.tile_pool(name="ps", bufs=7, space="PSUM"))
        psum_o = ctx.enter_context(tc.tile_pool(name="pso", bufs=1, space="PSUM"))

        # startup-critical loads first: fn (z-pass scalars, kt0 first), then
        # group 0's af broadcasts; bulk constants stream in behind them.
        fn_sb = consts.tile([128, 5, BL], F32)
        nc.sync.dma_start(out=fn_sb[:, 0, :], in_=fn_d[:, 0, :])
        wc_sb = consts.tile([128, R, 5, H], BF16)
        u_sb = consts.tile([128, T, 7], BF16)
        ft_sb = consts.tile([128, 5, BL], F32)
        wx_sb = consts.tile([128, 5, 7], F32)
        bx_sb = consts.tile([7, 1], F32)
        hm_all = hmp.tile([128, BL * T], BF16)

        # variable group sizes: tiny leading groups start the ACT pipeline
        # early (head latency is af-broadcast bound).
        SZ = [1, 3, 4, 10, 10, 10, 10, 8, 8]
        assert sum(SZ) == BL
        cum = 0
        op = None
        for g, nb_g in enumerate(SZ):
            b0 = cum
            cum += nb_g
            af_g = afp.tile([128, GB, K], BF16, tag="afg")
            for b in range(nb_g):
                # early batches gate the ACT pipeline start: split their
                # partition-broadcasts across queues for transfer parallelism
                nsplit = 4 if b0 + b == 0 else (2 if b0 + b < 4 else 1)
                step = 128 // nsplit
                for ci in range(nsplit):
                    eng = nc.sync if ci % 2 == 0 else nc.gpsimd
                    eng.dma_start(
                        out=af_g[ci * step : (ci + 1) * step, b, :],
                        in_=bass.AP(
                            tensor=af_d,
                            offset=(b0 + b) * K,
                            ap=[[0, step], [1, K]],
                        ),
                    )
            if g == 0:
                for kt in range(1, 5):
                    nc.sync.dma_start(out=fn_sb[:, kt, :], in_=fn_d[:, kt, :])
                nc.scalar.dma_start(out=wc_sb[:], in_=wc_d[:])
            if g == 2:
                nc.sync.dma_start(out=u_sb[:], in_=u_d[:])
                nc.sync.dma_start(out=ft_sb[:], in_=ft_d[:])
                nc.sync.dma_start(out=wx_sb[:], in_=wx_d[:])
                nc.sync.dma_start(out=bx_sb[:], in_=bx_d[:])
            atts = []
            for kt, (k0, kp) in enumerate(KTS):
                at = attp.tile([128, GB, K], BF16, tag=f"att{kt}")
                atts.append(at)
                if kt == 4:
                    nc.sync.dma_start(
                        out=at[88:89, 0:nb_g, :], in_=f_d[0:1, b0 : b0 + nb_g, :]
                    )
                for b in range(nb_g):
                    nc.vector.tensor_scalar_mul(
                        out=at[0:kp, b, :],
                        in0=af_g[0:kp, b, :],
                        scalar1=fn_sb[0:kp, kt, b0 + b : b0 + b + 1],
                    )
                nc.scalar.activation(
                    out=at[0:kp, 0:nb_g, :],
                    in_=at[0:kp, 0:nb_g, :],
                    func=mybir.ActivationFunctionType.Tanh,
                )
            chunks = [(s, min(3, nb_g - s)) for s in range(0, nb_g, 3)]
            ptiles = []
            for ci, (_, nb) in enumerate(chunks):
                pt = psum.tile([128, nb * T], F32, tag="hmps", padded_shape=[None, 512])
                ptiles.append(pt)
            for kt, (k0, kp) in enumerate(KTS):
                pp = kp + 1 if kt == 4 else kp
                for r in range(R):
                    lhsT = wc_sb[0:pp, r, kt, :]
                    for ci, (s, nb) in enumerate(chunks):
                        nc.tensor.matmul(
                            out=ptiles[ci][r * H : (r + 1) * H, 0 : nb * T],
                            lhsT=lhsT,
                            rhs=atts[kt][0:pp, s : s + nb, r * T : (r + 1) * T],
                            start=(kt == 0),
                            stop=(kt == 4),
                            tile_position=(0, r * H),
                            skip_group_check=True,
                        )
            for ci, (s, nb) in enumerate(chunks):
                nc.vector.tensor_scalar_max(
                    out=hm_all[:, (b0 + s) * T : (b0 + s + nb) * T],
                    in0=ptiles[ci][:, 0 : nb * T],
                    scalar1=0.0,
                )
            # final-output accumulation in two b-pieces: the first piece's
            # matmuls run while ACT is still busy with later groups.
            if (cum >= BL // 2 and op is None) or cum == BL:
                hm3 = hm_all.rearrange("p (b t) -> p b t", t=T)
                h0 = 0 if op is None else done_b
                hw = cum - h0
                done_b = cum
                if op is None:
                    op = psum_o.tile([7, BL], F32, padded_shape=[None, 512])
                for t in range(T):
                    nc.tensor.matmul(
                        out=op[:, h0 : h0 + hw],
                        lhsT=u_sb[:, t, :],
                        rhs=hm3[:, h0 : h0 + hw, t],
                        start=(t == 0),
                        stop=False,
                    )
                for kt, (k0, kp) in enumerate(KTS):
                    nc.tensor.matmul(
                        out=op[:, h0 : h0 + hw],
                        lhsT=wx_sb[0:kp, kt, :],
                        rhs=ft_sb[0:kp, kt, h0 : h0 + hw],
                        start=False,
                        stop=(kt == 4),
                    )

        ob = outp.tile([7, BL], F32)
        nc.vector.tensor_scalar_add(out=ob[:], in0=op[:], scalar1=bx_sb[:])
        nc.sync.dma_start(out=out_d[:], in_=ob[:])

    nc.finalize()
    return nc


def _host_prep(feats, a, W, Wc, Wh, W1, b1, W2, b2):
    """Per-core input maps. feats: [R,B,1,T] fp32."""
    f = feats[:, :, 0, :]                              # [R,B,T]
    af_full = a[:, None, None] * f                     # [R,B,T]
    feat_n = feats.reshape(B, K)                       # [B,K]
    Wx = W2 @ W1                                       # [7,K]
    bx = W2 @ b1 + b2                                  # [7]

    # U[(r,c), t, i] = Wh[r,c] * Wx[i, r*T+t]
    U = np.zeros((128, T, 7), np.float32)
    for r in range(R):
        blk = Wx[:, r * T : (r + 1) * T].T             # [T,7]
        U[r * H : (r + 1) * H] = Wh[r][:, None, None] * blk[None]

    # wc_h[p, r, kt, c]: Wc[r].T rows per k-tile; kt4 row 88 = W[r]
    wc_h = np.zeros((128, R, 5, H), np.float32)
    for r in range(R):
        for kt, (k0, kp) in enumerate(KTS):
            wc_h[:kp, r, kt, :] = Wc[r, :, k0 : k0 + kp].T
        wc_h[88, r, 4, :] = W[r]

    wx_h = np.zeros((128, 5, 7), np.float32)
    for kt, (k0, kp) in enumerate(KTS):
        wx_h[:kp, kt, :] = Wx[:, k0 : k0 + kp].T

    fT_full = np.concatenate([f[r].T for r in range(R)], axis=0)  # [K, B]

    in_maps = []
    for m in range(NCORES):
        b0 = m * BL
        af_h = np.ascontiguousarray(
            af_full[:, b0 : b0 + BL, :].transpose(1, 0, 2).reshape(BL, K)
        ).astype(BF)
        f_h = np.ascontiguousarray(
            f[:, b0 : b0 + BL, :].transpose(1, 0, 2).reshape(1, BL, K)
        ).astype(BF)
        fn_h = np.zeros((128, 5, BL), np.float32)
        for kt, (k0, kp) in enumerate(KTS):
            fn_h[:kp, kt, :] = feat_n[b0 : b0 + BL, k0 : k0 + kp].T
        ft_h = np.zeros((128, 5, BL), np.float32)
        for kt, (k0, kp) in enumerate(KTS):
            ft_h[:kp, kt, :] = fT_full[k0 : k0 + kp, b0 : b0 + BL]
        in_maps.append(
            {
                "af": af_h,
                "fr": f_h,
                "fn": fn_h,
                "wc": wc_h.astype(BF),
                "u": U.astype(BF),
                "ft": ft_h,
                "wx": wx_h,
                "bx": bx.astype(np.float32).reshape(7, 1),
            }
        )
    return in_maps


def kernel(feats_list, a, W, Wc, Wh, W1, b1, W2, b2):
    feats = np.asarray(feats_list, np.float32)
    in_maps = _host_prep(
        feats,
        np.asarray(a, np.float32),
        np.asarray(W, np.float32),
        np.asarray(Wc, np.float32),
        np.asarray(Wh, np.float32),
        np.asarray(W1, np.float32),
        np.asarray(b1, np.float32),
        np.asarray(W2, np.float32),
        np.asarray(b2, np.float32),
    )
    if "nc" not in _CACHE:
        _CACHE["nc"] = build_nc()
    res = bass_utils.run_bass_kernel_spmd(
        _CACHE["nc"], in_maps, core_ids=list(range(NCORES))
    )
    _CACHE["last_result"] = res
    out = np.concatenate([r["out"].T for r in res.results], axis=0)  # [B,7]
    return out[:, None, :].astype(np.float32)                        # [B,1,7]



# revision 12
# speedup vs baseline: 2.4304x; 2.4304x over previous
"""Trainium2 Bass kernel for nn_CAM_62852551409742 (low-rank tanh rewrite).

Math (reference):
  f = feats[:, :, 0, :]                               [R,B,T], R=4, B=512, T=150
  v = feats.reshape(B, K)                             [B,K], K=600
  att[r,b,t,k] = tanh(u[r,b,t] * v[b,k]),  u = a[r]*f
  Hm = relu(att @ Wc[r].T + f*W[r])                   [R,B,T,32]
  attf = Hm @ Wh[r] + f
  out = (attf-cat @ W1.T + b1) @ W2.T + b2            [B,1,7]

Key rewrite: |u*v| <= ~1.4 on this data, so tanh(uv) ~= sum_j c_j (uv)^p_j
(odd powers p=1..11, weighted-LS fit, err ~1e-5). Then
  att @ Wc.T = sum_j u^p_j * S_j,   S_j[r,b,c] = sum_k c_j v[b,k]^p_j Wc[r,c,k]
which kills the 184M-element tanh and the K=600 contraction entirely.

Per core (64 batches), bf16 operands with fp32 PSUM accumulation:
  stage D: S rows [32q+jf x (rc)] via PE (vp-packed strided lhsT, WcT rhs)
  stage A: per (b,r) col-tiled MMs  Hm[(rc),t] = sum_jf S_jf * phi_jf (+f*W row)
  relu:    DVE/ACT alternating, PSUM -> SBUF bf16
  final:   per-t MMs vs U[(rc),t,i]=Wh*Wx in 4 col groups + fp32 f@Wx tail + bias
"""

from contextlib import ExitStack

import numpy as np
import ml_dtypes

import concourse.bacc as bacc
import concourse.bass as bass
import concourse.tile as tile
from concourse import mybir
from concourse import bass_utils

R, B, T, H = 4, 512, 150, 32
K = R * T                      # 600
NCORES = 8
BL = B // NCORES               # 64 batches per core
J = 6                          # odd powers 1..11
JF = 7                         # 6 powers + f row
POWS = (1, 3, 5, 7, 9, 11)
KTS = [(0, 128), (128, 128), (256, 128), (384, 128), (512, 88)]
VPW = 416                      # per-kt packed width (384 used + overrun pad)
F32 = mybir.dt.float32
BF16 = mybir.dt.bfloat16
BF = ml_dtypes.bfloat16

_CACHE = {}


def build_nc():
    nc = bacc.Bacc("TRN2", target_bir_lowering=False)
    phi_d = nc.dram_tensor("phi", [JF, R * BL * T], BF16, kind="ExternalInput")
    vp_d = nc.dram_tensor("vp", [128, 5, 2048], BF16, kind="ExternalInput")
    wct_d = nc.dram_tensor("wct", [128, 5, 128], BF16, kind="ExternalInput")
    wr_d = nc.dram_tensor("wr", [1, 512], BF16, kind="ExternalInput")
    u_d = nc.dram_tensor("u", [128, T, 7], BF16, kind="ExternalInput")
    ft_d = nc.dram_tensor("ft", [128, 5, BL], F32, kind="ExternalInput")
    wx_d = nc.dram_tensor("wx", [128, 5, 7], F32, kind="ExternalInput")
    bx_d = nc.dram_tensor("bx", [7, 1], F32, kind="ExternalInput")
    out_d = nc.dram_tensor("out", [7, BL], F32, kind="ExternalOutput")

    with tile.TileContext(nc) as tc, ExitStack() as ctx:
        consts = ctx.enter_context(tc.tile_pool(name="consts", bufs=1))
        hmp = ctx.enter_context(tc.tile_pool(name="hm", bufs=1))
        ps_s = ctx.enter_context(tc.tile_pool(name="pss", bufs=2, space="PSUM"))
        ps_hm = ctx.enter_context(tc.tile_pool(name="psh", bufs=4, space="PSUM"))
        ps_o = ctx.enter_context(tc.tile_pool(name="pso", bufs=1, space="PSUM"))

        vp_full = consts.tile([128, 5, 2048], BF16)
        wct_sb = consts.tile([128, 5, 128], BF16)
        phi_sb = consts.tile([128, R * BL * T], BF16)
        s_all = consts.tile([128, 2048], BF16)
        u_sb = consts.tile([128, T, 7], BF16)
        ft_sb = consts.tile([128, 5, BL], F32)
        wx_sb = consts.tile([128, 5, 7], F32)
        bx_sb = consts.tile([7, 1], F32)
        hm_sb = hmp.tile([128, BL * T], BF16)
        tiny = consts.tile([1, 1], F32)

        # startup DMAs; stage-D inputs first, then phi blocks, then the rest.
        # phi basis rows go to all four partition blocks straight from HBM.
        nc.scalar.dma_start(out=wct_sb[:], in_=wct_d[:])
        nc.scalar.dma_start(out=vp_full[:, 0, :], in_=vp_d[:, 0, :])
        nc.sync.dma_start(out=phi_sb[0:JF, :], in_=phi_d[:])
        nc.gpsimd.dma_start(out=phi_sb[32:32 + JF, :], in_=phi_d[:])
        nc.sync.dma_start(out=phi_sb[64:64 + JF, :], in_=phi_d[:])
        nc.gpsimd.dma_start(out=phi_sb[96:96 + JF, :], in_=phi_d[:])
        for kt in range(1, 5):
            nc.scalar.dma_start(out=vp_full[:, kt, :], in_=vp_d[:, kt, :])
        # preload ACT's table set (has Relu) while DMAs run
        nc.vector.memset(tiny[:], 0.0)
        nc.scalar.activation(out=tiny[:], in_=tiny[:],
                             func=mybir.ActivationFunctionType.Relu)
        nc.gpsimd.dma_start(out=u_sb[:], in_=u_d[:])
        nc.gpsimd.dma_start(out=ft_sb[:], in_=ft_d[:])
        nc.gpsimd.dma_start(out=wx_sb[:], in_=wx_d[:])
        nc.gpsimd.dma_start(out=bx_sb[:], in_=bx_d[:])

        hm3 = hm_sb.rearrange("p (b t) -> p b t", t=T)

        def stage_d_group(g):
            """Produce S rows for chunks 4g..4g+3 (b = 16g..16g+15)."""
            sp = ps_s.tile([128, 512], F32, tag="sps")
            # chunk-serial: a start=True clears has_written for the WHOLE
            # bank, so interleaving chunks' accumulation groups loses data
            for mm in range(4):
                m = 4 * g + mm
                for kt, (k0, kp) in enumerate(KTS):
                    nc.tensor.matmul(
                        out=sp[:, 128 * mm:128 * (mm + 1)],
                        lhsT=vp_full[0:kp, kt, 128 * m:128 * (m + 1)],
                        rhs=wct_sb[0:kp, kt, :],
                        start=(kt == 0),
                        stop=(kt == 4),
                        skip_group_check=True,
                    )
            nc.vector.tensor_copy(s_all[:, 512 * g:512 * (g + 1)], sp[:])
            # overwrite rows 32q+6 with W (the f*W basis row)
            nc.gpsimd.dma_start(
                out=s_all[6:128:32, 512 * g:512 * (g + 1)],
                in_=bass.AP(tensor=wr_d, offset=0, ap=[[0, 4], [1, 512]]),
            )

        state = {"flip": False}

        def stage_a(bs):
            """Hm for a chunk of up to 3 batches (same q, consecutive m)."""
            pt = ps_hm.tile([128, 512], F32, tag="hmps")
            for slot, b in enumerate(bs):
                q, m = b % 4, b // 4
                for r in range(R):
                    nc.tensor.matmul(
                        out=pt[32 * r:32 * (r + 1),
                               150 * slot:150 * slot + 150],
                        lhsT=s_all[32 * q:32 * q + JF,
                                   128 * m + 32 * r:128 * m + 32 * r + 32],
                        rhs=phi_sb[32 * q:32 * q + JF,
                                   (r * BL + b) * T:(r * BL + b) * T + T],
                        start=True, stop=True,
                        tile_position=(32 * q, 32 * r),
                        skip_group_check=True,
                    )
            # relu chunk: PSUM fp32 -> SBUF bf16, strided over b (step 4)
            o = hm3[:, bs[0]:bs[-1] + 1:4, :]
            state["flip"] = not state["flip"]
            if state["flip"]:
                nc.vector.tensor_scalar_max(
                    out=o, in0=pt[:, 0:150 * len(bs)], scalar1=0.0)
            else:
                nc.scalar.activation(
                    out=o, in_=pt[:, 0:150 * len(bs)],
                    func=mybir.ActivationFunctionType.Relu)

        def q_pass(q, m_range):
            for i in range(0, len(m_range), 3):
                ms = m_range[i:i + 3]
                stage_a([4 * m + q for m in ms])

        # q-major batch order: q=0 interleaved with stage D, then q=1,2,3
        for g in range(4):
            stage_d_group(g)
            q_pass(0, list(range(4 * g, 4 * g + 4)))
        for q in range(1, 4):
            q_pass(q, list(range(16)))

        # final contraction: out[i,b] = sum_{(rc),t} relu(Hm)*U + sum f*Wx + bx
        op = ps_o.tile([128, BL], F32, padded_shape=[None, 512])
        glast = {gg: max(t for t in range(T) if t % 4 == gg) for gg in range(4)}
        for t in range(T):
            g = t % 4
            nc.tensor.matmul(
                out=op[32 * g:32 * g + 7, :],
                lhsT=u_sb[:, t, :],
                rhs=hm3[:, :, t],
                start=(t < 4),
                stop=(g != 0 and t == glast[g]),
                tile_position=(0, 32 * g),
                skip_group_check=True,
            )
        for kt, (k0, kp) in enumerate(KTS):
            nc.tensor.matmul(
                out=op[0:7, :],
                lhsT=wx_sb[0:kp, kt, :],
                rhs=ft_sb[0:kp, kt, :],
                start=False, stop=(kt == 4),
                tile_position=(0, 0),
                skip_group_check=True,
            )

        # tail: sum the 4 col groups + bias
        c1 = consts.tile([7, BL], F32)
        c2 = consts.tile([7, BL], F32)
        s1 = consts.tile([7, BL], F32)
        s2 = consts.tile([7, BL], F32)
        ob = consts.tile([7, BL], F32)
        nc.vector.tensor_copy(c1[:], op[32:39, :])
        nc.scalar.copy(c2[:], op[96:103, :])
        nc.vector.scalar_tensor_tensor(
            out=s1[:], in0=op[0:7, :], scalar=bx_sb[:], in1=c1[:],
            op0=mybir.AluOpType.add, op1=mybir.AluOpType.add)
        nc.vector.scalar_tensor_tensor(
            out=s2[:], in0=op[64:71, :], scalar=0.0, in1=c2[:],
            op0=mybir.AluOpType.add, op1=mybir.AluOpType.add)
        nc.vector.tensor_add(ob[:], s1[:], s2[:])
        nc.sync.dma_start(out=out_d[:], in_=ob[:])

    nc.finalize()
    return nc


def _fit_poly(u, v):
    xmax = float(np.abs(u).max()) * float(np.abs(v).max()) * 1.02 + 1e-30
    xs = xmax * np.sin(np.linspace(-np.pi / 2, np.pi / 2, 4001))
    A = xs[:, None] ** np.array(POWS)[None, :]
    w = 1.0 / (0.05 + np.abs(xs))
    coef, *_ = np.linalg.lstsq(A * w[:, None], np.tanh(xs) * w, rcond=None)
    return coef


def _host_prep(feats, a, W, Wc, Wh, W1, b1, W2, b2):
    f = feats[:, :, 0, :]                              # [R,B,T]
    u = a[:, None, None] * f                           # [R,B,T]
    v = feats.reshape(B, K)                            # [B,K]
    coef = _fit_poly(u, v)
    Wx = W2 @ W1                                       # [7,K]
    bx = W2 @ b1 + b2                                  # [7]

    # U[(rc), t, i] = Wh[r,c] * Wx[i, r*T+t]
    U = np.zeros((128, T, 7), np.float32)
    for r in range(R):
        blk = Wx[:, r * T:(r + 1) * T].T               # [T,7]
        U[r * H:(r + 1) * H] = Wh[r][:, None, None] * blk[None]

    # wct[k, kt, 32r+c] = Wc[r, c, k0+k]
    wct = np.zeros((128, 5, 128), np.float32)
    for kt, (k0, kp) in enumerate(KTS):
        for r in range(R):
            wct[:kp, kt, 32 * r:32 * (r + 1)] = Wc[r, :, k0:k0 + kp].T

    wr = np.tile(W.reshape(1, 128), (1, 4)).astype(BF)   # [1, 512]

    wx_h = np.zeros((128, 5, 7), np.float32)
    for kt, (k0, kp) in enumerate(KTS):
        wx_h[:kp, kt, :] = Wx[:, k0:k0 + kp].T

    fT_full = np.concatenate([f[r].T for r in range(R)], axis=0)  # [K, B]

    # basis powers with the tanh-poly coefficients folded into the v side
    vbasis = np.stack([coef[j] * v ** POWS[j] for j in range(J)], 0)  # [J,B,K]
    ubasis = np.stack([u ** POWS[j] for j in range(J)], 0)            # [J,R,B,T]

    in_maps = []
    for mcore in range(NCORES):
        b0 = mcore * BL
        # phi[jf, (r*BL+bl)*T + t]: jf<J -> u^p, jf=J -> f
        phi = np.zeros((JF, R * BL * T), np.float32)
        phi[0:J] = ubasis[:, :, b0:b0 + BL, :].reshape(J, R * BL * T)
        phi[J] = f[:, b0:b0 + BL, :].reshape(R * BL * T)
        # vp full width: [k, kt, m*128 + q*32 + jf] = vbasis[jf, b0+4m+q, k0+k]
        vp = np.zeros((128, 5, 2048), np.float32)
        for kt, (k0, kp) in enumerate(KTS):
            vb = vbasis[:, b0:b0 + BL, k0:k0 + kp]     # [J, BL, kp]
            arr = vb.transpose(2, 1, 0).reshape(kp, 16, 4, J)
            full = np.zeros((kp, 16, 4, 32), np.float32)
            full[..., :J] = arr
            vp[:kp, kt, :] = full.reshape(kp, 2048)
        ft_h = np.zeros((128, 5, BL), np.float32)
        for kt, (k0, kp) in enumerate(KTS):
            ft_h[:kp, kt, :] = fT_full[k0:k0 + kp, b0:b0 + BL]
        in_maps.append({
            "phi": phi.astype(BF),
            "vp": vp.astype(BF),
            "wct": wct.astype(BF),
            "wr": wr,
            "u": U.astype(BF),
            "ft": ft_h,
            "wx": wx_h,
            "bx": bx.astype(np.float32).reshape(7, 1),
        })
    return in_maps


def kernel(feats_list, a, W, Wc, Wh, W1, b1, W2, b2):
    feats = np.asarray(feats_list, np.float32)
    in_maps = _host_prep(
        feats,
        np.asarray(a, np.float32),
        np.asarray(W, np.float32),
        np.asarray(Wc, np.float32),
        np.asarray(Wh, np.float32),
        np.asarray(W1, np.float32),
        np.asarray(b1, np.float32),
        np.asarray(W2, np.float32),
        np.asarray(b2, np.float32),
    )
    if "nc" not in _CACHE:
        _CACHE["nc"] = build_nc()
    res = bass_utils.run_bass_kernel_spmd(
        _CACHE["nc"], in_maps, core_ids=list(range(NCORES))
    )
    _CACHE["last_result"] = res
    out = np.concatenate([r["out"].T for r in res.results], axis=0)  # [B,7]
    return out[:, None, :].astype(np.float32)                        # [B,1,7]
